# revision 7
# baseline (speedup 1.0000x reference)
"""Trainium2 Bass kernel for the 3-layer GAT (nn_GAT_56341380989571).

Strategy (8 NeuronCores, SPMD):
  - Nodes padded to 100352 and sharded contiguously: core k owns 12544 nodes
    (98 blocks of 128). Edges partitioned by receiver; within each core,
    edges are grouped per 128-node block and sorted by sender-quarter so a
    fixed number of 128-edge tiles per (block, quarter) can be gathered with
    int16-indexed dma_gather from quarter slices of the node table.
  - Per layer: each core computes its shard of the node table
    [hp | al_s | al_d] in bf16 (512B rows), AllGathers it, then gathers
    per-edge rows by sender, computes attention weights
    w = exp(leakyrelu(al_s[s]+al_d[r])) with the softmax max-subtraction
    dropped (logits are tiny) and the normalization folded to node level:
    agg[v] = sum_e w_e*hp[s_e] / (sum_e w_e + 1e-16).
  - Segment sums by receiver use an indicator matmul: tiles of 128 edges map
    to one 128-node block; IND[e,p] = (r_rel[e]==p) built by DVE is_equal,
    accumulated on PSUM over the block's tiles. al_d is expanded edge-wise
    via transpose(IND) @ al_d_block on the PE.
  - Graph mean-pool via a mask matmul accumulated during layer 3, followed by
    an AllReduce of the [100,128] partial sums and a redundant tiny MLP.
"""
import os
import sys
import types

import numpy as np

import concourse.bass as bass
import concourse.bacc as bacc
import concourse.mybir as mybir
import concourse.tile as tile
from concourse.bass_utils import run_bass_kernel_spmd
from concourse.library_config import mlp as _mlp_lib

F32 = mybir.dt.float32
BF16 = mybir.dt.bfloat16
I16 = mybir.dt.int16
AF = mybir.ActivationFunctionType
OP = mybir.AluOpType

# problem constants (hardcoded per spec)
N, E, G = 100000, 800000, 100
F_IN, DIM, H, L = 64, 128, 8, 3
HD = DIM // H
SLOPE = 0.2
NCORES = 8
BLK = 128
BPC = 98                 # blocks per core
NPC = BPC * BLK          # 12544 nodes per core
NPAD = NCORES * NPC      # 100352
NQ = NPAD // 4           # 25088 rows per table quarter
ELEM = 256               # bf16 elems per table row (512B)
GPB = 7                  # blocks per gather group
NGRP = BPC // GPB        # 14 groups
TPQ = 3                  # tiles per (block, quarter)
TPB = 4 * TPQ            # 12 tiles per block
NIDX = GPB * TPQ * 128   # 2688 idxs per gather call
NCALLS = NGRP * 4

last_exec_time_ns = None


def _build_program():
    nc = bacc.Bacc("TRN2", target_bir_lowering=False)

    # ---- DRAM I/O ----
    d_xT = nc.dram_tensor("xT", [F_IN, NPC], F32, kind="ExternalInput")
    d_win = nc.dram_tensor("win", [F_IN, DIM], F32, kind="ExternalInput")
    d_bin = nc.dram_tensor("bin", [DIM, 1], F32, kind="ExternalInput")
    d_wcat = nc.dram_tensor("wcat", [DIM, L * 144], F32, kind="ExternalInput")
    d_wskip = nc.dram_tensor("wskip", [DIM, L * DIM], BF16, kind="ExternalInput")
    d_bskip = nc.dram_tensor("bskip", [DIM, L], F32, kind="ExternalInput")
    d_idx = nc.dram_tensor("idx", [128, NCALLS * (NIDX // 16)], I16, kind="ExternalInput")
    d_rrel = nc.dram_tensor("rrel", [128, NGRP * GPB * TPB], BF16, kind="ExternalInput")
    d_msk = nc.dram_tensor("msk", [128, BPC * 100], BF16, kind="ExternalInput")
    d_iota = nc.dram_tensor("iota", [128, 128], BF16, kind="ExternalInput")
    d_eyebf = nc.dram_tensor("eyebf", [128, 128], BF16, kind="ExternalInput")
    d_eye32 = nc.dram_tensor("eye32", [128, 128], F32, kind="ExternalInput")
    d_w1 = nc.dram_tensor("w1", [DIM, DIM], F32, kind="ExternalInput")
    d_w2 = nc.dram_tensor("w2", [DIM, DIM], F32, kind="ExternalInput")
    d_w3 = nc.dram_tensor("w3", [DIM, 1], F32, kind="ExternalInput")
    d_b1 = nc.dram_tensor("b1b", [128, DIM], F32, kind="ExternalInput")
    d_b2 = nc.dram_tensor("b2b", [128, DIM], F32, kind="ExternalInput")
    d_b3 = nc.dram_tensor("b3b", [128, 1], F32, kind="ExternalInput")
    d_invn = nc.dram_tensor("invn", [128, 1], F32, kind="ExternalInput")
    d_out = nc.dram_tensor("out", [100, 1], F32, kind="ExternalOutput")

    with tile.TileContext(nc) as tc:
        with (
            tc.tile_pool(name="dram", bufs=1, space="DRAM") as dram,
            tc.tile_pool(name="cst", bufs=1) as cst,
            tc.tile_pool(name="gp", bufs=2) as gp,
            tc.tile_pool(name="wk", bufs=2) as wk,
            tc.tile_pool(name="xs", bufs=2) as xs,
            tc.tile_pool(name="pt", bufs=2, space="PSUM") as pt,
            tc.tile_pool(name="pa", bufs=2, space="PSUM") as pa,
            tc.tile_pool(name="pl", bufs=2, space="PSUM") as pl,
            tc.tile_pool(name="ph", bufs=1, space="PSUM") as ph,
            tc.tile_pool(name="pp", bufs=1, space="PSUM") as pp,
        ):
            # ---- persistent SBUF ----
            hT = cst.tile([128, NPC], F32, tag="hT")
            idx_sb = cst.tile([128, NCALLS * (NIDX // 16)], I16, tag="idx")
            rrel_sb = cst.tile([128, NGRP * GPB * TPB], BF16, tag="rrel")
            iota_sb = cst.tile([128, 128], BF16, tag="iota")
            eyebf_sb = cst.tile([128, 128], BF16, tag="eyebf")
            eye32_sb = cst.tile([128, 128], F32, tag="eye32")
            win_sb = cst.tile([F_IN, DIM], F32, tag="win")
            bin_sb = cst.tile([128, 1], F32, tag="bin")
            wcat_sb = cst.tile([128, L * 144], F32, tag="wcat")
            wskip_sb = cst.tile([128, L * DIM], BF16, tag="wskip")
            bskip_sb = cst.tile([128, L], F32, tag="bskip")
            ald_sb = cst.tile([128, BPC * 8], BF16, tag="ald")

            nc.sync.dma_start(idx_sb[:], d_idx[:])
            nc.sync.dma_start(rrel_sb[:], d_rrel[:])
            nc.sync.dma_start(iota_sb[:], d_iota[:])
            nc.sync.dma_start(eyebf_sb[:], d_eyebf[:])
            nc.sync.dma_start(eye32_sb[:], d_eye32[:])
            nc.sync.dma_start(win_sb[:], d_win[:])
            nc.sync.dma_start(bin_sb[:], d_bin[:])
            nc.sync.dma_start(wcat_sb[:], d_wcat[:])
            nc.sync.dma_start(wskip_sb[:], d_wskip[:])
            nc.sync.dma_start(bskip_sb[:], d_bskip[:])

            nc.gpsimd.load_library(_mlp_lib)

            # ---- DRAM tiles ----
            tab_in = dram.tile([NPC, ELEM], BF16, tag="tab_in")
            tab_out = dram.tile([NPAD, ELEM], BF16, tag="tab_out")
            ar_in = dram.tile([100, DIM], F32, tag="ar_in")
            ar_out = dram.tile([100, DIM], F32, tag="ar_out")

            # ---- stage 0: h0 = relu(x @ W_in + b_in), stored transposed ----
            CH0 = 512
            nch = NPC // CH0 + (1 if NPC % CH0 else 0)
            for c in range(nch):
                lo = c * CH0
                w = min(CH0, NPC - lo)
                xc = xs.tile([F_IN, CH0], F32, tag="xc")
                nc.sync.dma_start(xc[:, :w], d_xT[:, lo:lo + w])
                p0 = pa.tile([128, CH0], F32, tag="acc")
                nc.tensor.matmul(p0[:, :w], lhsT=win_sb[:], rhs=xc[:, :w],
                                 start=True, stop=True)
                nc.scalar.activation(hT[:, lo:lo + w], p0[:, :w], AF.Relu,
                                     bias=bin_sb[:, 0:1])

            # ---- layers ----
            pooled_ps = None
            for i in range(L):
                # table stage
                for c in range(BPC):
                    lo = c * BLK
                    pc = pa.tile([128, 144], F32, tag="acc")
                    nc.tensor.matmul(pc[:], lhsT=hT[:, lo:lo + BLK],
                                     rhs=wcat_sb[:, i * 144:(i + 1) * 144],
                                     start=True, stop=True)
                    tb = wk.tile([128, 144], BF16, tag="tab")
                    nc.vector.tensor_copy(tb[:], pc[:])
                    nc.vector.tensor_copy(ald_sb[:, c * 8:(c + 1) * 8],
                                          tb[:, 136:144])
                    nc.sync.dma_start(tab_in[lo:lo + BLK, 0:144], tb[:])
                # allgather the table
                nc.gpsimd.collective_compute(
                    "AllGather", OP.bypass,
                    ins=[tab_in.opt()], outs=[tab_out.opt()],
                    replica_groups=[list(range(NCORES))],
                )
                if i == L - 1:
                    pooled_ps = pp.tile([128, DIM], F32, tag="pool")
                # edge stage
                for g in range(NGRP):
                    Gt = gp.tile([128, 4 * GPB * TPQ * ELEM], BF16, tag="G")
                    G3 = Gt[:].rearrange("p (k c) -> p k c", c=ELEM)
                    for q in range(4):
                        call = g * 4 + q
                        nc.gpsimd.dma_gather(
                            G3[:, q * GPB * TPQ:(q + 1) * GPB * TPQ, :],
                            tab_out[q * NQ:(q + 1) * NQ, :],
                            idx_sb[:, call * (NIDX // 16):(call + 1) * (NIDX // 16)],
                            NIDX, NIDX, ELEM, single_packet=False,
                        )
                    for bg in range(GPB):
                        b = g * GPB + bg
                        blo = b * BLK
                        # indicator IND[e, p] for the block's 12 tiles
                        IND = wk.tile([128, TPB * 128], BF16, tag="IND")
                        ind4 = IND[:].rearrange("p (q j e) -> p q j e", q=4, e=128)
                        rr = rrel_sb[:].rearrange("p (g q b j) -> p g q b j",
                                                  g=NGRP, q=4, b=GPB)
                        in0 = rr[:, g, :, bg, :].unsqueeze(-1).broadcast_to([128, 4, TPQ, 128])
                        in1 = iota_sb[:].unsqueeze(1).unsqueeze(1).broadcast_to([128, 4, TPQ, 128])
                        nc.vector.tensor_tensor(out=ind4, in0=in0, in1=in1,
                                                op=OP.is_equal)
                        # al_d expansion via transpose(IND) @ al_d_block
                        ind2 = wk.tile([128, TPB * 128], BF16, tag="ind2")
                        pald = pl.tile([128, TPB * 8], F32, tag="ald")
                        for k in range(TPB):
                            ptk = pt.tile([128, 128], BF16, tag="pt")
                            nc.tensor.transpose(ptk[:], IND[:, k * 128:(k + 1) * 128],
                                                eyebf_sb[:])
                            nc.any.tensor_copy(ind2[:, k * 128:(k + 1) * 128], ptk[:])
                            nc.tensor.matmul(pald[:, k * 8:(k + 1) * 8],
                                             lhsT=ind2[:, k * 128:(k + 1) * 128],
                                             rhs=ald_sb[:, b * 8:(b + 1) * 8],
                                             start=True, stop=True)
                        # logits = al_s[s] + al_d[r]; w = exp(lrelu(logits))
                        Lg = wk.tile([128, TPB * 8], F32, tag="Lg")
                        l4 = Lg[:].rearrange("p (q j h) -> p q j h", q=4, h=8)
                        gals = G3[:, :, 128:136].rearrange(
                            "p (q b j) h -> p q b j h", q=4, b=GPB)[:, :, bg, :, :]
                        pald4 = pald[:].rearrange("p (q j h) -> p q j h", q=4, h=8)
                        nc.vector.tensor_tensor(out=l4, in0=pald4, in1=gals, op=OP.add)
                        Lm = wk.tile([128, TPB * 8], F32, tag="Lm")
                        nc.vector.tensor_scalar_mul(Lm[:], Lg[:], SLOPE)
                        nc.vector.tensor_tensor(out=Lm[:], in0=Lg[:], in1=Lm[:], op=OP.max)
                        R = wk.tile([128, TPB * 136], BF16, tag="R")
                        R3 = R[:].rearrange("p (k c) -> p k c", c=136)
                        nc.scalar.activation(R3[:, :, 128:136],
                                             Lm[:].rearrange("p (k h) -> p k h", h=8),
                                             AF.Exp)
                        # contrib = hp * w (per quarter to keep APs <= 4D)
                        for q in range(4):
                            ghp = G3[:, q * GPB * TPQ + bg * TPQ:
                                     q * GPB * TPQ + (bg + 1) * TPQ, 0:128]
                            ghp = ghp.rearrange("p j (h d) -> p j h d", d=HD)
                            rw = R3[:, q * TPQ:(q + 1) * TPQ, 128:136]
                            rw = rw.unsqueeze(-1).broadcast_to([128, TPQ, 8, HD])
                            rc = R3[:, q * TPQ:(q + 1) * TPQ, 0:128]
                            rc = rc.rearrange("p j (h d) -> p j h d", d=HD)
                            nc.vector.tensor_tensor(out=rc, in0=ghp, in1=rw, op=OP.mult)
                        # segment matmuls (accumulate over the block's tiles)
                        pagg = pa.tile([128, 144], F32, tag="acc")
                        for k in range(TPB):
                            nc.tensor.matmul(pagg[:, 0:136],
                                             lhsT=IND[:, k * 128:(k + 1) * 128],
                                             rhs=R[:, k * 136:(k + 1) * 136],
                                             start=(k == 0), stop=(k == TPB - 1))
                        # normalize + skip + residual + leaky relu
                        rec = wk.tile([128, 8], F32, tag="rec")
                        nc.vector.tensor_scalar_add(rec[:], pagg[:, 128:136], 1e-16)
                        nc.vector.reciprocal(rec[:], rec[:])
                        aggn = wk.tile([128, 128], BF16, tag="aggn")
                        a3 = aggn[:].rearrange("p (h d) -> p h d", d=HD)
                        nc.vector.tensor_tensor(
                            out=a3,
                            in0=pagg[:, 0:128].rearrange("p (h d) -> p h d", d=HD),
                            in1=rec[:].unsqueeze(-1).broadcast_to([128, 8, HD]),
                            op=OP.mult)
                        ptn = pt.tile([128, 128], BF16, tag="pt")
                        nc.tensor.transpose(ptn[:], aggn[:], eyebf_sb[:])
                        aggT = wk.tile([128, 128], BF16, tag="aggT")
                        nc.any.tensor_copy(aggT[:], ptn[:])
                        phd = ph.tile([128, 128], F32, tag="hd")
                        nc.tensor.matmul(phd[:], lhsT=wskip_sb[:, i * DIM:(i + 1) * DIM],
                                         rhs=aggT[:], start=True, stop=True)
                        tmp = wk.tile([128, 128], F32, tag="tmp")
                        nc.vector.tensor_tensor(out=tmp[:], in0=hT[:, blo:blo + BLK],
                                                in1=phd[:], op=OP.add)
                        nc.vector.tensor_scalar_add(tmp[:], tmp[:], bskip_sb[:, i:i + 1])
                        tmp2 = wk.tile([128, 128], F32, tag="tmp2")
                        nc.vector.tensor_scalar_mul(tmp2[:], tmp[:], SLOPE)
                        nc.vector.tensor_tensor(out=hT[:, blo:blo + BLK], in0=tmp[:],
                                                in1=tmp2[:], op=OP.max)
                        if i == L - 1:
                            # pooled[g, d] += Msk_blk.T @ h_rows_blk
                            ptr = pt.tile([128, 128], F32, tag="pt")
                            nc.tensor.transpose(ptr[:], hT[:, blo:blo + BLK], eye32_sb[:])
                            hrow = wk.tile([128, 128], BF16, tag="hrow")
                            nc.any.tensor_copy(hrow[:], ptr[:])
                            mskb = wk.tile([128, 100], BF16, tag="mskb")
                            nc.sync.dma_start(mskb[:], d_msk[:, b * 100:(b + 1) * 100])
                            nc.tensor.matmul(pooled_ps[:100, :], lhsT=mskb[:],
                                             rhs=hrow[:], start=(b == 0),
                                             stop=(b == BPC - 1),
                                             skip_group_check=True)

            # ---- pooling allreduce + MLP ----
            pooled_sb = cst.tile([128, DIM], F32, tag="pooled")
            nc.vector.memset(pooled_sb[:], 0.0)
            nc.vector.tensor_copy(pooled_sb[:100, :], pooled_ps[:100, :])
            nc.sync.dma_start(ar_in[:], pooled_sb[:100, :])
            nc.gpsimd.collective_compute(
                "AllReduce", OP.add,
                ins=[ar_in.opt()], outs=[ar_out.opt()],
                replica_groups=[list(range(NCORES))],
            )
            nc.sync.dma_start(pooled_sb[:100, :], ar_out[:])
            invn_sb = cst.tile([128, 1], F32, tag="invn")
            nc.sync.dma_start(invn_sb[:], d_invn[:])
            nc.vector.tensor_scalar_mul(pooled_sb[:], pooled_sb[:], invn_sb[:, 0:1])

            w1_sb = cst.tile([128, DIM], F32, tag="w1")
            w2_sb = cst.tile([128, DIM], F32, tag="w2")
            w3_sb = cst.tile([128, 1], F32, tag="w3")
            b1_sb = cst.tile([128, DIM], F32, tag="b1")
            b2_sb = cst.tile([128, DIM], F32, tag="b2")
            b3_sb = cst.tile([128, 1], F32, tag="b3")
            nc.sync.dma_start(w1_sb[:], d_w1[:])
            nc.sync.dma_start(w2_sb[:], d_w2[:])
            nc.sync.dma_start(w3_sb[:], d_w3[:])
            nc.sync.dma_start(b1_sb[:], d_b1[:])
            nc.sync.dma_start(b2_sb[:], d_b2[:])
            nc.sync.dma_start(b3_sb[:], d_b3[:])

            def mlp_layer(src_sb, w_sb, b_sb, ncols):
                # z = lrelu(src @ W + b) computed via transpose + matmul
                ptz = pt.tile([128, 128], F32, tag="pt")
                nc.tensor.transpose(ptz[:], src_sb[:], eye32_sb[:])
                srcT = wk.tile([128, 128], F32, tag="srcT")
                nc.vector.tensor_copy(srcT[:], ptz[:])
                pz = pa.tile([128, 144], F32, tag="acc")
                nc.tensor.matmul(pz[:100, :ncols], lhsT=srcT[:, 0:100],
                                 rhs=w_sb[:, :ncols], start=True, stop=True)
                zo = wk.tile([128, DIM], F32, tag="zo")
                nc.vector.memset(zo[:], 0.0)
                nc.vector.tensor_tensor(out=zo[:100, :ncols], in0=pz[:100, :ncols],
                                        in1=b_sb[:100, :ncols], op=OP.add)
                z2 = wk.tile([128, DIM], F32, tag="z2")
                nc.vector.memset(z2[:], 0.0)
                nc.vector.tensor_scalar_mul(z2[:100, :ncols], zo[:100, :ncols], SLOPE)
                nc.vector.tensor_tensor(out=zo[:100, :ncols], in0=zo[:100, :ncols],
                                        in1=z2[:100, :ncols], op=OP.max)
                return zo

            z1 = mlp_layer(pooled_sb, w1_sb, b1_sb, DIM)
            z1k = cst.tile([128, DIM], F32, tag="z1k")
            nc.vector.tensor_copy(z1k[:], z1[:])
            z2 = mlp_layer(z1k, w2_sb, b2_sb, DIM)
            z2k = cst.tile([128, DIM], F32, tag="z2k")
            nc.vector.tensor_copy(z2k[:], z2[:])
            # final: out = z2 @ W3 + b3  (no lrelu)
            ptz = pt.tile([128, 128], F32, tag="pt")
            nc.tensor.transpose(ptz[:], z2k[:], eye32_sb[:])
            zT = wk.tile([128, 128], F32, tag="srcT")
            nc.vector.tensor_copy(zT[:], ptz[:])
            po = pa.tile([128, 144], F32, tag="acc")
            nc.tensor.matmul(po[:100, 0:1], lhsT=zT[:, 0:100], rhs=w3_sb[:],
                             start=True, stop=True)
            outp = cst.tile([128, 1], F32, tag="outp")
            nc.vector.tensor_tensor(out=outp[:100, :], in0=po[:100, 0:1],
                                    in1=b3_sb[:100, :], op=OP.add)
            nc.sync.dma_start(d_out[:], outp[:100, :])

    nc.compile()
    return nc


def _wrap_idx(flat):
    """Lay out int16 gather indices in the Q7 wrap layout for one call."""
    n = flat.shape[0]
    arr = np.zeros((16, n // 16), np.int16)
    ii = np.arange(n)
    arr[ii % 16, ii // 16] = flat.astype(np.int16)
    return np.tile(arr, (8, 1))


def _preprocess(x, senders, receivers, n_node):
    """Build per-core input arrays."""
    order = np.argsort(receivers, kind="stable")
    r_s = receivers[order].astype(np.int64)
    s_s = senders[order].astype(np.int64)
    quarter = s_s // NQ

    graph_of = np.full(NPAD, -1, np.int64)
    graph_of[:N] = np.repeat(np.arange(G), n_node.astype(np.int64))

    per_core = []
    for c in range(NCORES):
        lo, hi = c * NPC, (c + 1) * NPC
        m = (r_s >= lo) & (r_s < hi)
        rc, sc, qc = r_s[m], s_s[m], quarter[m]
        blk = (rc - lo) // BLK
        # order edges by (block, quarter) stably
        key = blk * 4 + qc
        o2 = np.argsort(key, kind="stable")
        rc, sc, qc, blk = rc[o2], sc[o2], qc[o2], blk[o2]
        # slot layout: per (block, quarter) capacity TPQ*128, grouped per
        # gather call (g, q): blocks of the group concatenated.
        idx_arr = np.zeros((128, NCALLS * (NIDX // 16)), np.int16)
        rrel_arr = np.full((128, NGRP * GPB * TPB), 128.0, np.float32)
        # bucket edges
        cap = TPQ * 128
        start_of = {}
        counts = np.zeros((BPC, 4), np.int64)
        for bq in range(BPC * 4):
            bb, qq = bq // 4, bq % 4
            sel = (blk == bb) & (qc == qq)
            cnt = int(sel.sum())
            if cnt > cap:
                raise RuntimeError(f"(block,quarter) capacity exceeded: {cnt} > {cap}")
            counts[bb, qq] = cnt
            start_of[(bb, qq)] = sel
        for g in range(NGRP):
            for q in range(4):
                call = g * 4 + q
                flat = np.zeros(NIDX, np.int64)  # local row in quarter (pad -> 0)
                for bg in range(GPB):
                    bb = g * GPB + bg
                    sel = start_of[(bb, q)]
                    cnt = counts[bb, q]
                    base = bg * cap
                    if cnt:
                        flat[base:base + cnt] = sc[sel] - q * NQ
                        rloc = rc[sel] - lo - bb * BLK
                        # tile index within group layout + rrel
                        for t in range(TPQ):
                            a, bnd = t * 128, min((t + 1) * 128, cnt)
                            if a >= cnt:
                                break
                            colg = g * GPB * TPB + q * GPB * TPQ + bg * TPQ + t
                            rrel_arr[0:bnd - a, colg] = rloc[a:bnd]
                idx_arr[:, call * (NIDX // 16):(call + 1) * (NIDX // 16)] = _wrap_idx(flat)
        # pooling mask
        msk = np.zeros((128, BPC * 100), np.float32)
        nodes = np.arange(lo, hi)
        gg = graph_of[nodes].reshape(BPC, BLK)
        for bb in range(BPC):
            valid = gg[bb] >= 0
            msk[np.arange(BLK)[valid], bb * 100 + gg[bb][valid]] = 1.0
        # xT shard
        xT = np.zeros((F_IN, NPC), np.float32)
        nreal = max(0, min(NPC, N - lo))
        if nreal > 0:
            xT[:, :nreal] = x[lo:lo + nreal].T
        per_core.append(dict(
            xT=xT,
            idx=idx_arr,
            rrel=rrel_arr.astype("bfloat16"),
            msk=msk.astype("bfloat16"),
        ))
    return per_core


def _rrel_layout_fix(rrel_arr):
    return rrel_arr


def kernel(**inputs):
    global last_exec_time_ns
    x = np.asarray(inputs["x"], np.float32)
    senders = np.asarray(inputs["senders"])
    receivers = np.asarray(inputs["receivers"])
    n_node = np.asarray(inputs["n_node"])

    per_core = _preprocess(x, senders, receivers, n_node)

    # shared weights
    W_in = np.asarray(inputs["W_in"], np.float32)
    b_in = np.asarray(inputs["b_in"], np.float32)
    W_gat = np.asarray(inputs["W_gat"], np.float32)
    a_src = np.asarray(inputs["a_src"], np.float32)
    a_dst = np.asarray(inputs["a_dst"], np.float32)
    W_skip = np.asarray(inputs["W_skip"], np.float32)
    b_skip = np.asarray(inputs["b_skip"], np.float32)
    W1 = np.asarray(inputs["W1"], np.float32)
    b1 = np.asarray(inputs["b1"], np.float32)
    W2 = np.asarray(inputs["W2"], np.float32)
    b2 = np.asarray(inputs["b2"], np.float32)
    W3 = np.asarray(inputs["W3"], np.float32)
    b3 = np.asarray(inputs["b3"], np.float32)

    def w_al(Wg, a):
        A = np.zeros((DIM, H), np.float32)
        for hh in range(H):
            A[hh * HD:(hh + 1) * HD, hh] = a[hh]
        return Wg @ A

    wcat = np.concatenate(
        [np.concatenate([W_gat[i], w_al(W_gat[i], a_src[i]),
                         w_al(W_gat[i], a_dst[i])], axis=1) for i in range(L)],
        axis=1)  # [128, L*144]
    wskip = np.concatenate([W_skip[i] for i in range(L)], axis=1).astype("bfloat16")
    bskip = np.stack([b_skip[i] for i in range(L)], axis=1)  # [128, L]

    iota = np.tile(np.arange(128, dtype=np.float32), (128, 1)).astype("bfloat16")
    eyebf = np.eye(128, dtype=np.float32).astype("bfloat16")
    eye32 = np.eye(128, dtype=np.float32)
    b1b = np.tile(b1, (128, 1)).astype(np.float32)
    b2b = np.tile(b2, (128, 1)).astype(np.float32)
    b3b = np.full((128, 1), float(b3[0]), np.float32)
    invn = np.ones((128, 1), np.float32)
    invn[:100, 0] = 1.0 / n_node.astype(np.float32)

    shared = dict(
        win=W_in, bin=b_in.reshape(DIM, 1), wcat=wcat, wskip=wskip, bskip=bskip,
        iota=iota, eyebf=eyebf, eye32=eye32,
        w1=W1, w2=W2, w3=W3.reshape(DIM, 1), b1b=b1b, b2b=b2b, b3b=b3b, invn=invn,
    )

    nc = _build_program()
    in_maps = [{**shared, **pc} for pc in per_core]
    trace = bool(int(os.environ.get("GAT_TRACE", "0")))
    res = run_bass_kernel_spmd(nc, in_maps, core_ids=list(range(NCORES)),
                               trace=trace)
    last_exec_time_ns = res.exec_time_ns
    out = np.asarray(res.results[0]["out"], np.float32).reshape(-1)
    return out


# revision 10
# speedup vs baseline: 1.6241x; 1.6241x over previous
"""Trainium2 Bass kernel for the 3-layer GAT (nn_GAT_56341380989571).

Strategy (8 NeuronCores, SPMD):
  - Nodes padded to 100352 and sharded contiguously: core k owns 12544 nodes
    (98 blocks of 128). Edges partitioned by receiver; within each core,
    edges are grouped per 128-node block and sorted by sender-quarter so a
    fixed number of 128-edge tiles per (block, quarter) can be gathered with
    int16-indexed dma_gather from quarter slices of the node table.
  - Per layer: each core computes its shard of the node table
    [hp | al_s | al_d] in bf16 (512B rows), AllGathers it, then gathers
    per-edge rows by sender, computes attention weights
    w = exp(leakyrelu(al_s[s]+al_d[r])) with the softmax max-subtraction
    dropped (logits are tiny) and the normalization folded to node level:
    agg[v] = sum_e w_e*hp[s_e] / (sum_e w_e + 1e-16).
  - Segment sums by receiver use an indicator matmul: tiles of 128 edges map
    to one 128-node block; IND[e,p] = (r_rel[e]==p) built by DVE is_equal,
    accumulated on PSUM over the block's tiles. al_d is expanded edge-wise
    via transpose(IND) @ al_d_block on the PE.
  - Graph mean-pool via a mask matmul accumulated during layer 3, followed by
    an AllReduce of the [100,128] partial sums and a redundant tiny MLP.
"""
import os
import sys
import types

import numpy as np

import concourse.bass as bass
import concourse.bacc as bacc
import concourse.mybir as mybir
import concourse.tile as tile
from concourse.bass_utils import run_bass_kernel_spmd
from concourse.library_config import mlp as _mlp_lib

F32 = mybir.dt.float32
BF16 = mybir.dt.bfloat16
I16 = mybir.dt.int16
AF = mybir.ActivationFunctionType
OP = mybir.AluOpType

# problem constants (hardcoded per spec)
N, E, G = 100000, 800000, 100
F_IN, DIM, H, L = 64, 128, 8, 3
HD = DIM // H
SLOPE = 0.2
NCORES = 8
BLK = 128
BPC = 98                 # blocks per core
NPC = BPC * BLK          # 12544 nodes per core
NPAD = NCORES * NPC      # 100352
NQ = NPAD // 4           # 25088 rows per table quarter
ELEM = 256               # bf16 elems per table row (512B)
GPB = 7                  # blocks per gather group
NGRP = BPC // GPB        # 14 groups
TPQ = 3                  # tiles per (block, quarter)
TPB = 4 * TPQ            # 12 tiles per block
NIDX = GPB * TPQ * 128   # 2688 idxs per gather call
NCALLS = NGRP * 4

last_exec_time_ns = None


def _build_program():
    nc = bacc.Bacc("TRN2", target_bir_lowering=False, num_swdge_queues=4)

    # ---- DRAM I/O ----
    d_xT = nc.dram_tensor("xT", [F_IN, NPC], F32, kind="ExternalInput")
    d_win = nc.dram_tensor("win", [F_IN, DIM], F32, kind="ExternalInput")
    d_bin = nc.dram_tensor("bin", [DIM, 1], F32, kind="ExternalInput")
    d_wcat = nc.dram_tensor("wcat", [DIM, L * 144], F32, kind="ExternalInput")
    d_wskip = nc.dram_tensor("wskip", [DIM, L * DIM], BF16, kind="ExternalInput")
    d_bskip = nc.dram_tensor("bskip", [DIM, L], F32, kind="ExternalInput")
    d_idx = nc.dram_tensor("idx", [128, NCALLS * (NIDX // 16)], I16, kind="ExternalInput")
    d_rrel = nc.dram_tensor("rrel", [128, NGRP * GPB * TPB], BF16, kind="ExternalInput")
    d_msk = nc.dram_tensor("msk", [128, BPC * 100], BF16, kind="ExternalInput")
    d_iota = nc.dram_tensor("iota", [128, 128], BF16, kind="ExternalInput")
    d_eyebf = nc.dram_tensor("eyebf", [128, 128], BF16, kind="ExternalInput")
    d_eye32 = nc.dram_tensor("eye32", [128, 128], F32, kind="ExternalInput")
    d_w1 = nc.dram_tensor("w1", [DIM, DIM], F32, kind="ExternalInput")
    d_w2 = nc.dram_tensor("w2", [DIM, DIM], F32, kind="ExternalInput")
    d_w3 = nc.dram_tensor("w3", [DIM, 1], F32, kind="ExternalInput")
    d_b1 = nc.dram_tensor("b1b", [128, DIM], F32, kind="ExternalInput")
    d_b2 = nc.dram_tensor("b2b", [128, DIM], F32, kind="ExternalInput")
    d_b3 = nc.dram_tensor("b3b", [128, 1], F32, kind="ExternalInput")
    d_invn = nc.dram_tensor("invn", [128, 1], F32, kind="ExternalInput")
    d_out = nc.dram_tensor("out", [100, 1], F32, kind="ExternalOutput")

    with tile.TileContext(nc) as tc:
        with (
            tc.tile_pool(name="dram", bufs=1, space="DRAM") as dram,
            tc.tile_pool(name="cst", bufs=1) as cst,
            tc.tile_pool(name="gp", bufs=2) as gp,
            tc.tile_pool(name="wk", bufs=2) as wk,
            tc.tile_pool(name="xs", bufs=2) as xs,
            tc.tile_pool(name="pt", bufs=2, space="PSUM") as pt,
            tc.tile_pool(name="pa", bufs=2, space="PSUM") as pa,
            tc.tile_pool(name="pl", bufs=2, space="PSUM") as pl,
            tc.tile_pool(name="ph", bufs=1, space="PSUM") as ph,
            tc.tile_pool(name="pp", bufs=1, space="PSUM") as pp,
        ):
            # ---- persistent SBUF ----
            hT = cst.tile([128, NPC], F32, tag="hT")
            idx_sb = cst.tile([128, NCALLS * (NIDX // 16)], I16, tag="idx")
            rrel_sb = cst.tile([128, NGRP * GPB * TPB], BF16, tag="rrel")
            iota_sb = cst.tile([128, 128], BF16, tag="iota")
            eyebf_sb = cst.tile([128, 128], BF16, tag="eyebf")
            eye32_sb = cst.tile([128, 128], F32, tag="eye32")
            win_sb = cst.tile([F_IN, DIM], F32, tag="win")
            bin_sb = cst.tile([128, 1], F32, tag="bin")
            wcat_sb = cst.tile([128, L * 144], F32, tag="wcat")
            wskip_sb = cst.tile([128, L * DIM], BF16, tag="wskip")
            bskip_sb = cst.tile([128, L], F32, tag="bskip")
            ald_sb = cst.tile([128, BPC * 8], BF16, tag="ald")

            nc.sync.dma_start(idx_sb[:], d_idx[:])
            nc.sync.dma_start(rrel_sb[:], d_rrel[:])
            nc.sync.dma_start(iota_sb[:], d_iota[:])
            nc.sync.dma_start(eyebf_sb[:], d_eyebf[:])
            nc.sync.dma_start(eye32_sb[:], d_eye32[:])
            nc.sync.dma_start(win_sb[:], d_win[:])
            nc.sync.dma_start(bin_sb[:], d_bin[:])
            nc.sync.dma_start(wcat_sb[:], d_wcat[:])
            nc.sync.dma_start(wskip_sb[:], d_wskip[:])
            nc.sync.dma_start(bskip_sb[:], d_bskip[:])

            nc.gpsimd.load_library(_mlp_lib)

            # ---- DRAM tiles ----
            tab_in = dram.tile([NPC, ELEM], BF16, tag="tab_in")
            tab_outs = [dram.tile([NPAD, ELEM], BF16, tag=f"tab_out{j}", name=f"tab_out{j}",
                                  addr_space="Shared") for j in range(L)]
            ar_in = dram.tile([100, DIM], F32, tag="ar_in")
            ar_out = dram.tile([100, DIM], F32, tag="ar_out", addr_space="Shared")

            # ---- stage 0: h0 = relu(x @ W_in + b_in), stored transposed ----
            CH0 = 512
            nch = NPC // CH0 + (1 if NPC % CH0 else 0)
            for c in range(nch):
                lo = c * CH0
                w = min(CH0, NPC - lo)
                xc = xs.tile([F_IN, CH0], F32, tag="xc")
                nc.sync.dma_start(xc[:, :w], d_xT[:, lo:lo + w])
                p0 = pa.tile([128, CH0], F32, tag="acc")
                nc.tensor.matmul(p0[:, :w], lhsT=win_sb[:], rhs=xc[:, :w],
                                 start=True, stop=True)
                nc.scalar.activation(hT[:, lo:lo + w], p0[:, :w], AF.Relu,
                                     bias=bin_sb[:, 0:1])

            # ---- layers ----
            pooled_ps = None
            for i in range(L):
                # table stage
                for c in range(BPC):
                    lo = c * BLK
                    pc = pa.tile([128, 144], F32, tag="acc")
                    nc.tensor.matmul(pc[:], lhsT=hT[:, lo:lo + BLK],
                                     rhs=wcat_sb[:, i * 144:(i + 1) * 144],
                                     start=True, stop=True)
                    tb = wk.tile([128, 144], BF16, tag="tab")
                    nc.scalar.activation(tb[:], pc[:], AF.Copy)
                    nc.vector.tensor_copy(ald_sb[:, c * 8:(c + 1) * 8],
                                          tb[:, 136:144])
                    nc.sync.dma_start(tab_in[lo:lo + BLK, 0:144], tb[:])
                # allgather the table
                tab_out = tab_outs[i]
                nc.gpsimd.collective_compute(
                    "AllGather", OP.bypass,
                    ins=[tab_in.opt()], outs=[tab_out.opt()],
                    replica_groups=[list(range(NCORES))],
                )
                if i == L - 1:
                    pooled_ps = pp.tile([128, DIM], F32, tag="pool")
                # edge stage
                for g in range(NGRP):
                    Gt = gp.tile([128, 4 * GPB * TPQ * ELEM], BF16, tag="G")
                    G3 = Gt[:].rearrange("p (k c) -> p k c", c=ELEM)
                    for q in range(4):
                        call = g * 4 + q
                        nc.gpsimd.dma_gather(
                            G3[:, q * GPB * TPQ:(q + 1) * GPB * TPQ, :],
                            tab_out[q * NQ:(q + 1) * NQ, :],
                            idx_sb[:, call * (NIDX // 16):(call + 1) * (NIDX // 16)],
                            NIDX, NIDX, ELEM, single_packet=False, queue_num=q,
                        )
                    for bg in range(GPB):
                        b = g * GPB + bg
                        blo = b * BLK
                        # indicator IND[e, p] for the block's 12 tiles
                        IND = wk.tile([128, TPB * 128], BF16, tag="IND")
                        ind4 = IND[:].rearrange("p (q j e) -> p q j e", q=4, e=128)
                        rr = rrel_sb[:].rearrange("p (g q b j) -> p g q b j",
                                                  g=NGRP, q=4, b=GPB)
                        in0 = rr[:, g, :, bg, :].unsqueeze(-1).broadcast_to([128, 4, TPQ, 128])
                        in1 = iota_sb[:].unsqueeze(1).unsqueeze(1).broadcast_to([128, 4, TPQ, 128])
                        nc.vector.tensor_tensor(out=ind4, in0=in0, in1=in1,
                                                op=OP.is_equal)
                        # al_d expansion via transpose(IND) @ al_d_block
                        ind2 = wk.tile([128, TPB * 128], BF16, tag="ind2")
                        pald = pl.tile([128, TPB * 8], F32, tag="ald")
                        for kk in range(TPB // 4):
                            ptk = pt.tile([128, 512], BF16, tag="pt")
                            for j4 in range(4):
                                k = kk * 4 + j4
                                nc.tensor.transpose(
                                    ptk[:, j4 * 128:(j4 + 1) * 128],
                                    IND[:, k * 128:(k + 1) * 128], eyebf_sb[:])
                            nc.scalar.activation(ind2[:, kk * 512:(kk + 1) * 512],
                                                 ptk[:], AF.Copy)
                        for k in range(TPB):
                            nc.tensor.matmul(pald[:, k * 8:(k + 1) * 8],
                                             lhsT=ind2[:, k * 128:(k + 1) * 128],
                                             rhs=ald_sb[:, b * 8:(b + 1) * 8],
                                             start=True, stop=True)
                        # logits = al_s[s] + al_d[r]; w = exp(lrelu(logits))
                        Lg = wk.tile([128, TPB * 8], F32, tag="Lg")
                        l4 = Lg[:].rearrange("p (q j h) -> p q j h", q=4, h=8)
                        gals = G3[:, :, 128:136].rearrange(
                            "p (q b j) h -> p q b j h", q=4, b=GPB)[:, :, bg, :, :]
                        pald4 = pald[:].rearrange("p (q j h) -> p q j h", q=4, h=8)
                        nc.vector.tensor_tensor(out=l4, in0=pald4, in1=gals, op=OP.add)
                        Lm = wk.tile([128, TPB * 8], F32, tag="Lm")
                        nc.vector.tensor_scalar_mul(Lm[:], Lg[:], SLOPE)
                        nc.vector.tensor_tensor(out=Lm[:], in0=Lg[:], in1=Lm[:], op=OP.max)
                        R = wk.tile([128, TPB * 136], BF16, tag="R")
                        R3 = R[:].rearrange("p (k c) -> p k c", c=136)
                        nc.scalar.activation(R3[:, :, 128:136],
                                             Lm[:].rearrange("p (k h) -> p k h", h=8),
                                             AF.Exp)
                        # contrib = hp * w (per quarter to keep APs <= 4D)
                        for q in range(4):
                            ghp = G3[:, q * GPB * TPQ + bg * TPQ:
                                     q * GPB * TPQ + (bg + 1) * TPQ, 0:128]
                            ghp = ghp.rearrange("p j (h d) -> p j h d", d=HD)
                            rw = R3[:, q * TPQ:(q + 1) * TPQ, 128:136]
                            rw = rw.unsqueeze(-1).broadcast_to([128, TPQ, 8, HD])
                            rc = R3[:, q * TPQ:(q + 1) * TPQ, 0:128]
                            rc = rc.rearrange("p j (h d) -> p j h d", d=HD)
                            nc.vector.tensor_tensor(out=rc, in0=ghp, in1=rw, op=OP.mult)
                        # segment matmuls (accumulate over the block's tiles)
                        pagg = pa.tile([128, 144], F32, tag="acc")
                        for k in range(TPB):
                            nc.tensor.matmul(pagg[:, 0:136],
                                             lhsT=IND[:, k * 128:(k + 1) * 128],
                                             rhs=R[:, k * 136:(k + 1) * 136],
                                             start=(k == 0), stop=(k == TPB - 1))
                        # normalize + skip + residual + leaky relu
                        rec = wk.tile([128, 8], F32, tag="rec")
                        nc.vector.tensor_scalar_add(rec[:], pagg[:, 128:136], 1e-16)
                        nc.vector.reciprocal(rec[:], rec[:])
                        aggn = wk.tile([128, 128], BF16, tag="aggn")
                        a3 = aggn[:].rearrange("p (h d) -> p h d", d=HD)
                        nc.vector.tensor_tensor(
                            out=a3,
                            in0=pagg[:, 0:128].rearrange("p (h d) -> p h d", d=HD),
                            in1=rec[:].unsqueeze(-1).broadcast_to([128, 8, HD]),
                            op=OP.mult)
                        ptn = pt.tile([128, 128], BF16, tag="pt")
                        nc.tensor.transpose(ptn[:], aggn[:], eyebf_sb[:])
                        aggT = wk.tile([128, 128], BF16, tag="aggT")
                        nc.scalar.activation(aggT[:], ptn[:], AF.Copy)
                        phd = ph.tile([128, 128], F32, tag="hd")
                        nc.tensor.matmul(phd[:], lhsT=wskip_sb[:, i * DIM:(i + 1) * DIM],
                                         rhs=aggT[:], start=True, stop=True)
                        tmp = wk.tile([128, 128], F32, tag="tmp")
                        nc.vector.tensor_tensor(out=tmp[:], in0=hT[:, blo:blo + BLK],
                                                in1=phd[:], op=OP.add)
                        nc.vector.tensor_scalar_add(tmp[:], tmp[:], bskip_sb[:, i:i + 1])
                        tmp2 = wk.tile([128, 128], F32, tag="tmp2")
                        nc.vector.tensor_scalar_mul(tmp2[:], tmp[:], SLOPE)
                        nc.vector.tensor_tensor(out=hT[:, blo:blo + BLK], in0=tmp[:],
                                                in1=tmp2[:], op=OP.max)
                        if i == L - 1:
                            # pooled[g, d] += Msk_blk.T @ h_rows_blk
                            ptr = pt.tile([128, 128], F32, tag="pt")
                            nc.tensor.transpose(ptr[:], hT[:, blo:blo + BLK], eye32_sb[:])
                            hrow = wk.tile([128, 128], BF16, tag="hrow")
                            nc.scalar.activation(hrow[:], ptr[:], AF.Copy)
                            mskb = wk.tile([128, 100], BF16, tag="mskb")
                            nc.sync.dma_start(mskb[:], d_msk[:, b * 100:(b + 1) * 100])
                            nc.tensor.matmul(pooled_ps[:100, :], lhsT=mskb[:],
                                             rhs=hrow[:], start=(b == 0),
                                             stop=(b == BPC - 1),
                                             skip_group_check=True)

            # ---- pooling allreduce + MLP ----
            pooled_sb = cst.tile([128, DIM], F32, tag="pooled")
            nc.vector.memset(pooled_sb[:], 0.0)
            nc.vector.tensor_copy(pooled_sb[:100, :], pooled_ps[:100, :])
            nc.sync.dma_start(ar_in[:], pooled_sb[:100, :])
            nc.gpsimd.collective_compute(
                "AllReduce", OP.add,
                ins=[ar_in.opt()], outs=[ar_out.opt()],
                replica_groups=[list(range(NCORES))],
            )
            nc.sync.dma_start(pooled_sb[:100, :], ar_out[:])
            invn_sb = cst.tile([128, 1], F32, tag="invn")
            nc.sync.dma_start(invn_sb[:], d_invn[:])
            nc.vector.tensor_scalar_mul(pooled_sb[:], pooled_sb[:], invn_sb[:, 0:1])

            w1_sb = cst.tile([128, DIM], F32, tag="w1")
            w2_sb = cst.tile([128, DIM], F32, tag="w2")
            w3_sb = cst.tile([128, 1], F32, tag="w3")
            b1_sb = cst.tile([128, DIM], F32, tag="b1")
            b2_sb = cst.tile([128, DIM], F32, tag="b2")
            b3_sb = cst.tile([128, 1], F32, tag="b3")
            nc.sync.dma_start(w1_sb[:], d_w1[:])
            nc.sync.dma_start(w2_sb[:], d_w2[:])
            nc.sync.dma_start(w3_sb[:], d_w3[:])
            nc.sync.dma_start(b1_sb[:], d_b1[:])
            nc.sync.dma_start(b2_sb[:], d_b2[:])
            nc.sync.dma_start(b3_sb[:], d_b3[:])

            def mlp_layer(src_sb, w_sb, b_sb, ncols):
                # z = lrelu(src @ W + b) computed via transpose + matmul
                ptz = pt.tile([128, 128], F32, tag="pt")
                nc.tensor.transpose(ptz[:], src_sb[:], eye32_sb[:])
                srcT = wk.tile([128, 128], F32, tag="srcT")
                nc.vector.tensor_copy(srcT[:], ptz[:])
                pz = pa.tile([128, 144], F32, tag="acc")
                nc.tensor.matmul(pz[:100, :ncols], lhsT=srcT[:, 0:100],
                                 rhs=w_sb[:, :ncols], start=True, stop=True)
                zo = wk.tile([128, DIM], F32, tag="zo")
                nc.vector.memset(zo[:], 0.0)
                nc.vector.tensor_tensor(out=zo[:100, :ncols], in0=pz[:100, :ncols],
                                        in1=b_sb[:100, :ncols], op=OP.add)
                z2 = wk.tile([128, DIM], F32, tag="z2")
                nc.vector.memset(z2[:], 0.0)
                nc.vector.tensor_scalar_mul(z2[:100, :ncols], zo[:100, :ncols], SLOPE)
                nc.vector.tensor_tensor(out=zo[:100, :ncols], in0=zo[:100, :ncols],
                                        in1=z2[:100, :ncols], op=OP.max)
                return zo

            z1 = mlp_layer(pooled_sb, w1_sb, b1_sb, DIM)
            z1k = cst.tile([128, DIM], F32, tag="z1k")
            nc.vector.tensor_copy(z1k[:], z1[:])
            z2 = mlp_layer(z1k, w2_sb, b2_sb, DIM)
            z2k = cst.tile([128, DIM], F32, tag="z2k")
            nc.vector.tensor_copy(z2k[:], z2[:])
            # final: out = z2 @ W3 + b3  (no lrelu)
            ptz = pt.tile([128, 128], F32, tag="pt")
            nc.tensor.transpose(ptz[:], z2k[:], eye32_sb[:])
            zT = wk.tile([128, 128], F32, tag="srcT")
            nc.vector.tensor_copy(zT[:], ptz[:])
            po = pa.tile([128, 144], F32, tag="acc")
            nc.tensor.matmul(po[:100, 0:1], lhsT=zT[:, 0:100], rhs=w3_sb[:],
                             start=True, stop=True)
            outp = cst.tile([128, 1], F32, tag="outp")
            nc.vector.tensor_tensor(out=outp[:100, :], in0=po[:100, 0:1],
                                    in1=b3_sb[:100, :], op=OP.add)
            nc.sync.dma_start(d_out[:], outp[:100, :])

    nc.compile()
    return nc


def _wrap_idx(flat):
    """Lay out int16 gather indices in the Q7 wrap layout for one call."""
    n = flat.shape[0]
    arr = np.zeros((16, n // 16), np.int16)
    ii = np.arange(n)
    arr[ii % 16, ii // 16] = flat.astype(np.int16)
    return np.tile(arr, (8, 1))


def _preprocess(x, senders, receivers, n_node):
    """Build per-core input arrays."""
    order = np.argsort(receivers, kind="stable")
    r_s = receivers[order].astype(np.int64)
    s_s = senders[order].astype(np.int64)
    quarter = s_s // NQ

    graph_of = np.full(NPAD, -1, np.int64)
    graph_of[:N] = np.repeat(np.arange(G), n_node.astype(np.int64))

    per_core = []
    for c in range(NCORES):
        lo, hi = c * NPC, (c + 1) * NPC
        m = (r_s >= lo) & (r_s < hi)
        rc, sc, qc = r_s[m], s_s[m], quarter[m]
        blk = (rc - lo) // BLK
        # order edges by (block, quarter) stably
        key = blk * 4 + qc
        o2 = np.argsort(key, kind="stable")
        rc, sc, qc, blk = rc[o2], sc[o2], qc[o2], blk[o2]
        # slot layout: per (block, quarter) capacity TPQ*128, grouped per
        # gather call (g, q): blocks of the group concatenated.
        idx_arr = np.zeros((128, NCALLS * (NIDX // 16)), np.int16)
        rrel_arr = np.full((128, NGRP * GPB * TPB), 128.0, np.float32)
        # bucket edges
        cap = TPQ * 128
        start_of = {}
        counts = np.zeros((BPC, 4), np.int64)
        for bq in range(BPC * 4):
            bb, qq = bq // 4, bq % 4
            sel = (blk == bb) & (qc == qq)
            cnt = int(sel.sum())
            if cnt > cap:
                raise RuntimeError(f"(block,quarter) capacity exceeded: {cnt} > {cap}")
            counts[bb, qq] = cnt
            start_of[(bb, qq)] = sel
        for g in range(NGRP):
            for q in range(4):
                call = g * 4 + q
                flat = np.zeros(NIDX, np.int64)  # local row in quarter (pad -> 0)
                for bg in range(GPB):
                    bb = g * GPB + bg
                    sel = start_of[(bb, q)]
                    cnt = counts[bb, q]
                    base = bg * cap
                    if cnt:
                        flat[base:base + cnt] = sc[sel] - q * NQ
                        rloc = rc[sel] - lo - bb * BLK
                        # tile index within group layout + rrel
                        for t in range(TPQ):
                            a, bnd = t * 128, min((t + 1) * 128, cnt)
                            if a >= cnt:
                                break
                            colg = g * GPB * TPB + q * GPB * TPQ + bg * TPQ + t
                            rrel_arr[0:bnd - a, colg] = rloc[a:bnd]
                idx_arr[:, call * (NIDX // 16):(call + 1) * (NIDX // 16)] = _wrap_idx(flat)
        # pooling mask
        msk = np.zeros((128, BPC * 100), np.float32)
        nodes = np.arange(lo, hi)
        gg = graph_of[nodes].reshape(BPC, BLK)
        for bb in range(BPC):
            valid = gg[bb] >= 0
            msk[np.arange(BLK)[valid], bb * 100 + gg[bb][valid]] = 1.0
        # xT shard
        xT = np.zeros((F_IN, NPC), np.float32)
        nreal = max(0, min(NPC, N - lo))
        if nreal > 0:
            xT[:, :nreal] = x[lo:lo + nreal].T
        per_core.append(dict(
            xT=xT,
            idx=idx_arr,
            rrel=rrel_arr.astype("bfloat16"),
            msk=msk.astype("bfloat16"),
        ))
    return per_core


def _rrel_layout_fix(rrel_arr):
    return rrel_arr


def kernel(**inputs):
    global last_exec_time_ns
    x = np.asarray(inputs["x"], np.float32)
    senders = np.asarray(inputs["senders"])
    receivers = np.asarray(inputs["receivers"])
    n_node = np.asarray(inputs["n_node"])

    per_core = _preprocess(x, senders, receivers, n_node)

    # shared weights
    W_in = np.asarray(inputs["W_in"], np.float32)
    b_in = np.asarray(inputs["b_in"], np.float32)
    W_gat = np.asarray(inputs["W_gat"], np.float32)
    a_src = np.asarray(inputs["a_src"], np.float32)
    a_dst = np.asarray(inputs["a_dst"], np.float32)
    W_skip = np.asarray(inputs["W_skip"], np.float32)
    b_skip = np.asarray(inputs["b_skip"], np.float32)
    W1 = np.asarray(inputs["W1"], np.float32)
    b1 = np.asarray(inputs["b1"], np.float32)
    W2 = np.asarray(inputs["W2"], np.float32)
    b2 = np.asarray(inputs["b2"], np.float32)
    W3 = np.asarray(inputs["W3"], np.float32)
    b3 = np.asarray(inputs["b3"], np.float32)

    def w_al(Wg, a):
        A = np.zeros((DIM, H), np.float32)
        for hh in range(H):
            A[hh * HD:(hh + 1) * HD, hh] = a[hh]
        return Wg @ A

    wcat = np.concatenate(
        [np.concatenate([W_gat[i], w_al(W_gat[i], a_src[i]),
                         w_al(W_gat[i], a_dst[i])], axis=1) for i in range(L)],
        axis=1)  # [128, L*144]
    wskip = np.concatenate([W_skip[i] for i in range(L)], axis=1).astype("bfloat16")
    bskip = np.stack([b_skip[i] for i in range(L)], axis=1)  # [128, L]

    iota = np.tile(np.arange(128, dtype=np.float32), (128, 1)).astype("bfloat16")
    eyebf = np.eye(128, dtype=np.float32).astype("bfloat16")
    eye32 = np.eye(128, dtype=np.float32)
    b1b = np.tile(b1, (128, 1)).astype(np.float32)
    b2b = np.tile(b2, (128, 1)).astype(np.float32)
    b3b = np.full((128, 1), float(b3[0]), np.float32)
    invn = np.ones((128, 1), np.float32)
    invn[:100, 0] = 1.0 / n_node.astype(np.float32)

    shared = dict(
        win=W_in, bin=b_in.reshape(DIM, 1), wcat=wcat, wskip=wskip, bskip=bskip,
        iota=iota, eyebf=eyebf, eye32=eye32,
        w1=W1, w2=W2, w3=W3.reshape(DIM, 1), b1b=b1b, b2b=b2b, b3b=b3b, invn=invn,
    )

    nc = _build_program()
    in_maps = [{**shared, **pc} for pc in per_core]
    trace = bool(int(os.environ.get("GAT_TRACE", "0")))
    res = run_bass_kernel_spmd(nc, in_maps, core_ids=list(range(NCORES)),
                               trace=trace)
    last_exec_time_ns = res.exec_time_ns
    out = np.asarray(res.results[0]["out"], np.float32).reshape(-1)
    return out


# revision 17
# speedup vs baseline: 1.8727x; 1.1531x over previous
"""Trainium2 Bass kernel for the 3-layer GAT (nn_GAT_56341380989571).

Strategy (8 NeuronCores, SPMD):
  - Nodes padded to 100352, sharded contiguously: core k owns 12544 nodes
    (98 blocks of 128). Edges partitioned by receiver; per core, edges are
    bucketed per (128-node block, sender-quarter) with fixed capacity
    3 tiles x 128 edges, so int16-indexed dma_gather calls (one per
    7-block group x quarter, spread over 4 SWDGE queues) fetch per-edge
    rows from quarter slices of the allgathered node table.
  - Per layer: each core computes its shard of the node table
    [hp | al_s | al_d] in bf16 (512B rows), AllGathers it (Shared DRAM),
    gathers rows by sender, computes attention weights
    w = exp(leakyrelu(al_s[s]+al_d[r])) with the softmax max-subtraction
    dropped (logits are tiny) and the normalization folded to node level:
    agg[v] = sum_e w_e*hp[s_e] / (sum_e w_e + 1e-16).
  - Segment sums by receiver via indicator matmuls; the indicator tiles
    IND[e,p] = (r_rel[e]==p) and their transposes IND2 are precomputed on
    the host (layer-invariant) and streamed from DRAM. al_d is expanded
    edge-wise as IND2 @ al_d_block on the PE.
  - Residual add folded into the PE (identity-matmul accumulation onto the
    skip matmul); bias + PSUM eviction on the scalar engine; leaky-relu on
    the vector engine. Blocks processed in pairs to halve small-op counts.
  - Graph mean-pool via a mask matmul accumulated during layer 3, then an
    AllReduce of [100,128] partial sums and a redundant tiny MLP.
"""
import os

import numpy as np

import concourse.bacc as bacc
import concourse.mybir as mybir
import concourse.tile as tile
from concourse.bass_utils import run_bass_kernel_spmd
from concourse.library_config import mlp as _mlp_lib

F32 = mybir.dt.float32
BF16 = mybir.dt.bfloat16
I16 = mybir.dt.int16
AF = mybir.ActivationFunctionType
OP = mybir.AluOpType

# problem constants (hardcoded per spec)
N, E, G = 100000, 800000, 100
F_IN, DIM, H, L = 64, 128, 8, 3
HD = DIM // H
SLOPE = 0.2
NCORES = 8
BLK = 128
BPC = 98                 # blocks per core
NPC = BPC * BLK          # 12544 nodes per core
NPAD = NCORES * NPC      # 100352
NQ = NPAD // 4           # 25088 rows per table quarter
ELEM = 256               # bf16 elems per table row (512B)
GPB = 7                  # blocks per gather group
NGRP = BPC // GPB        # 14 groups
TPQ = 3                  # tiles per (block, quarter)
TPB = 4 * TPQ            # 12 tiles per block
NIDX = GPB * TPQ * 128   # 2688 idxs per gather call
NCALLS = NGRP * 4
TT = BPC * TPB           # 1176 tiles per core

last_exec_time_ns = None


def _build_program():
    nc = bacc.Bacc("TRN2", target_bir_lowering=False, num_swdge_queues=4)

    # ---- DRAM I/O ----
    d_xT = nc.dram_tensor("xT", [F_IN, NPC], F32, kind="ExternalInput")
    d_win = nc.dram_tensor("win", [F_IN, DIM], F32, kind="ExternalInput")
    d_bin = nc.dram_tensor("bin", [DIM, 1], F32, kind="ExternalInput")
    d_wcat = nc.dram_tensor("wcat", [DIM, L * 144], F32, kind="ExternalInput")
    d_wskip = nc.dram_tensor("wskip", [DIM, L * DIM], BF16, kind="ExternalInput")
    d_bskip = nc.dram_tensor("bskip", [DIM, L], F32, kind="ExternalInput")
    d_idx = nc.dram_tensor("idx", [128, NCALLS * (NIDX // 16)], I16, kind="ExternalInput")
    d_ind = nc.dram_tensor("ind", [128, TT * 128], BF16, kind="ExternalInput")
    d_ind2 = nc.dram_tensor("ind2", [128, TT * 128], BF16, kind="ExternalInput")
    d_msk = nc.dram_tensor("msk", [128, BPC * 100], BF16, kind="ExternalInput")
    d_eyebf = nc.dram_tensor("eyebf", [128, 128], BF16, kind="ExternalInput")
    d_eye32 = nc.dram_tensor("eye32", [128, 128], F32, kind="ExternalInput")
    d_w1 = nc.dram_tensor("w1", [DIM, DIM], F32, kind="ExternalInput")
    d_w2 = nc.dram_tensor("w2", [DIM, DIM], F32, kind="ExternalInput")
    d_w3 = nc.dram_tensor("w3", [DIM, 1], F32, kind="ExternalInput")
    d_b1 = nc.dram_tensor("b1b", [128, DIM], F32, kind="ExternalInput")
    d_b2 = nc.dram_tensor("b2b", [128, DIM], F32, kind="ExternalInput")
    d_b3 = nc.dram_tensor("b3b", [128, 1], F32, kind="ExternalInput")
    d_invn = nc.dram_tensor("invn", [128, 1], F32, kind="ExternalInput")
    d_out = nc.dram_tensor("out", [100, 1], F32, kind="ExternalOutput")

    with tile.TileContext(nc) as tc:
        with (
            tc.tile_pool(name="dram", bufs=1, space="DRAM") as dram,
            tc.tile_pool(name="cst", bufs=1) as cst,
            tc.tile_pool(name="gp", bufs=2) as gp,
            tc.tile_pool(name="ip", bufs=2) as ip,
            tc.tile_pool(name="i2", bufs=1) as i2,
            tc.tile_pool(name="wk", bufs=2) as wk,
            tc.tile_pool(name="pt", bufs=2, space="PSUM") as pt,
            tc.tile_pool(name="pa", bufs=2, space="PSUM") as pa,
            tc.tile_pool(name="pl", bufs=2, space="PSUM") as pl,
            tc.tile_pool(name="ph", bufs=1, space="PSUM") as ph,
            tc.tile_pool(name="pp", bufs=1, space="PSUM") as pp,
        ):
            # ---- persistent SBUF ----
            hT = cst.tile([128, NPC], F32, tag="hT")
            idx_sb = cst.tile([128, NCALLS * (NIDX // 16)], I16, tag="idx")
            eyebf_sb = cst.tile([128, 128], BF16, tag="eyebf")
            eye32_sb = cst.tile([128, 128], F32, tag="eye32")
            win_sb = cst.tile([F_IN, DIM], F32, tag="win")
            bin_sb = cst.tile([128, 1], F32, tag="bin")
            wcat_sb = cst.tile([128, L * 144], F32, tag="wcat")
            wskip_sb = cst.tile([128, L * DIM], BF16, tag="wskip")
            bskip_sb = cst.tile([128, L], F32, tag="bskip")
            ald_sb = cst.tile([128, BPC * 8], BF16, tag="ald")

            nc.sync.dma_start(idx_sb[:], d_idx[:])
            nc.sync.dma_start(eyebf_sb[:], d_eyebf[:])
            nc.sync.dma_start(eye32_sb[:], d_eye32[:])
            nc.sync.dma_start(win_sb[:], d_win[:])
            nc.sync.dma_start(bin_sb[:], d_bin[:])
            nc.sync.dma_start(wcat_sb[:], d_wcat[:])
            nc.sync.dma_start(wskip_sb[:], d_wskip[:])
            nc.sync.dma_start(bskip_sb[:], d_bskip[:])

            nc.gpsimd.load_library(_mlp_lib)

            # ---- DRAM tiles ----
            tab_in = dram.tile([NPC, ELEM], BF16, tag="tab_in")
            tab_outs = [dram.tile([NPAD, ELEM], BF16, tag=f"tab_out{j}",
                                  name=f"tab_out{j}", addr_space="Shared")
                        for j in range(L)]
            ar_in = dram.tile([100, DIM], F32, tag="ar_in")
            ar_out = dram.tile([100, DIM], F32, tag="ar_out", addr_space="Shared")

            # ---- stage 0: h0 = relu(x @ W_in + b_in), stored transposed ----
            CH0 = 256
            nch = NPC // CH0 + (1 if NPC % CH0 else 0)
            for c in range(nch):
                lo = c * CH0
                w = min(CH0, NPC - lo)
                xc = ip.tile([F_IN, CH0], F32, tag="xc")
                nc.sync.dma_start(xc[:, :w], d_xT[:, lo:lo + w])
                p0 = pa.tile([128, CH0], F32, tag="acc")
                nc.tensor.matmul(p0[:, :w], lhsT=win_sb[:], rhs=xc[:, :w],
                                 start=True, stop=True)
                nc.scalar.activation(hT[:, lo:lo + w], p0[:, :w], AF.Relu,
                                     bias=bin_sb[:, 0:1])

            # ---- layers ----
            pooled_ps = None
            for i in range(L):
                # table stage
                for c in range(BPC):
                    lo = c * BLK
                    pc = pa.tile([128, 144], F32, tag="acc")
                    nc.tensor.matmul(pc[:], lhsT=hT[:, lo:lo + BLK],
                                     rhs=wcat_sb[:, i * 144:(i + 1) * 144],
                                     start=True, stop=True)
                    tb = wk.tile([128, 144], BF16, tag="tab")
                    nc.scalar.activation(tb[:], pc[:], AF.Copy)
                    nc.vector.tensor_copy(ald_sb[:, c * 8:(c + 1) * 8],
                                          tb[:, 136:144])
                    nc.sync.dma_start(tab_in[lo:lo + BLK, 0:144], tb[:])
                # allgather the table
                tab_out = tab_outs[i]
                nc.gpsimd.collective_compute(
                    "AllGather", OP.bypass,
                    ins=[tab_in.opt()], outs=[tab_out.opt()],
                    replica_groups=[list(range(NCORES))],
                )
                if i == L - 1:
                    pooled_ps = pp.tile([128, DIM], F32, tag="pool")
                # edge stage
                for g in range(NGRP):
                    Gt = gp.tile([128, 4 * GPB * TPQ * ELEM], BF16, tag="G")
                    G3 = Gt[:].rearrange("p (k c) -> p k c", c=ELEM)
                    for q in range(4):
                        call = g * 4 + q
                        nc.gpsimd.dma_gather(
                            G3[:, q * GPB * TPQ:(q + 1) * GPB * TPQ, :],
                            tab_out[q * NQ:(q + 1) * NQ, :],
                            idx_sb[:, call * (NIDX // 16):(call + 1) * (NIDX // 16)],
                            NIDX, NIDX, ELEM, single_packet=False, queue_num=q,
                        )
                    for bg0 in range(0, GPB, 2):
                        nb = min(2, GPB - bg0)        # 2 or 1 blocks this step
                        b0 = g * GPB + bg0
                        blo = b0 * BLK
                        ntile = nb * TPQ              # tiles per quarter chunk
                        npair = nb * TPB              # total tiles (24 or 12)
                        # stream indicator tiles (host-precomputed)
                        IND = ip.tile([128, 2 * TPB * 128], BF16, tag="IND")
                        IND2 = i2.tile([128, 2 * TPB * 128], BF16, tag="IND2")
                        icol = (b0 * TPB) * 128
                        nc.sync.dma_start(IND[:, :npair * 128],
                                          d_ind[:, icol:icol + npair * 128])
                        nc.sync.dma_start(IND2[:, :npair * 128],
                                          d_ind2[:, icol:icol + npair * 128])
                        # al_d expansion on PE (tile t of block bb)
                        pald = pl.tile([128, 2 * TPB * 8], F32, tag="ald")
                        for bb in range(nb):
                            for k in range(TPB):
                                kk = bb * TPB + k
                                nc.tensor.matmul(
                                    pald[:, kk * 8:(kk + 1) * 8],
                                    lhsT=IND2[:, kk * 128:(kk + 1) * 128],
                                    rhs=ald_sb[:, (b0 + bb) * 8:(b0 + bb + 1) * 8],
                                    start=True, stop=True)
                        # logits = al_s[s] + al_d[r]
                        Lg = wk.tile([128, 2 * TPB * 8], F32, tag="Lg")
                        for bb in range(nb):
                            gals = G3[:, :, 128:136].rearrange(
                                "p (q k) h -> p q k h", q=4)[
                                :, :, (bg0 + bb) * TPQ:(bg0 + bb + 1) * TPQ, :]
                            l4 = Lg[:, bb * 96:(bb + 1) * 96].rearrange(
                                "p (q k h) -> p q k h", q=4, h=8)
                            p4 = pald[:, bb * 96:(bb + 1) * 96].rearrange(
                                "p (q k h) -> p q k h", q=4, h=8)
                            nc.vector.tensor_tensor(out=l4, in0=p4, in1=gals,
                                                    op=OP.add)
                        # w = exp(lrelu(logits))
                        Lm = wk.tile([128, 2 * TPB * 8], F32, tag="Lm")
                        nc.vector.tensor_scalar_mul(Lm[:, :npair * 8],
                                                    Lg[:, :npair * 8], SLOPE)
                        nc.vector.tensor_tensor(out=Lm[:, :npair * 8],
                                                in0=Lg[:, :npair * 8],
                                                in1=Lm[:, :npair * 8], op=OP.max)
                        R = wk.tile([128, 2 * TPB * 136], BF16, tag="R")
                        R3 = R[:].rearrange("p (k c) -> p k c", c=136)
                        nc.scalar.activation(
                            R3[:, :npair, 128:136],
                            Lm[:, :npair * 8].rearrange("p (k h) -> p k h", h=8),
                            AF.Exp)
                        # contrib = hp * w  (per quarter; pair blocks contiguous)
                        for q in range(4):
                            for bb in range(nb):
                                ghp = G3[:, q * GPB * TPQ + (bg0 + bb) * TPQ:
                                         q * GPB * TPQ + (bg0 + bb + 1) * TPQ, 0:128]
                                ghp = ghp.rearrange("p k (h d) -> p k h d", d=HD)
                                ks = bb * TPB + q * TPQ
                                rsel = R3[:, ks:ks + TPQ, :]
                                rw = rsel[:, :, 128:136].unsqueeze(-1) \
                                    .broadcast_to([128, TPQ, 8, HD])
                                rc = rsel[:, :, 0:128].rearrange(
                                    "p k (h d) -> p k h d", d=HD)
                                nc.vector.tensor_tensor(out=rc, in0=ghp, in1=rw,
                                                        op=OP.mult)
                        # segment matmuls: one accumulator per block, same bank
                        pagg = pa.tile([128, 2 * 144], F32, tag="acc")
                        for bb in range(nb):
                            for k in range(TPB):
                                kk = bb * TPB + k
                                nc.tensor.matmul(
                                    pagg[:, bb * 144:bb * 144 + 136],
                                    lhsT=IND[:, kk * 128:(kk + 1) * 128],
                                    rhs=R[:, kk * 136:(kk + 1) * 136],
                                    start=(k == 0), stop=(k == TPB - 1))
                        # normalize
                        rec = wk.tile([128, 16], F32, tag="rec")
                        den = pagg[:].rearrange("p (b c) -> p b c", b=2)[
                            :, :nb, 128:136]
                        rec3 = rec[:, :nb * 8].rearrange("p (b c) -> p b c", b=nb)
                        nc.vector.tensor_scalar_add(rec3, den, 1e-16)
                        nc.vector.reciprocal(rec[:, :nb * 8], rec[:, :nb * 8])
                        aggn = wk.tile([128, 2 * 128], BF16, tag="aggn")
                        for bb in range(nb):
                            nc.vector.tensor_tensor(
                                out=aggn[:, bb * 128:(bb + 1) * 128].rearrange(
                                    "p (h d) -> p h d", d=HD),
                                in0=pagg[:, bb * 144:bb * 144 + 128].rearrange(
                                    "p (h d) -> p h d", d=HD),
                                in1=rec[:, bb * 8:(bb + 1) * 8].unsqueeze(-1)
                                    .broadcast_to([128, 8, HD]),
                                op=OP.mult)
                        # skip matmul + residual on PE; bias + evict on ACT
                        for bb in range(nb):
                            ptn = pt.tile([128, 128], BF16, tag="pt")
                            nc.tensor.transpose(
                                ptn[:], aggn[:, bb * 128:(bb + 1) * 128],
                                eyebf_sb[:])
                            aggT = wk.tile([128, 128], BF16, tag="aggT")
                            nc.scalar.activation(aggT[:], ptn[:], AF.Copy)
                            phd = ph.tile([128, 128], F32, tag="hd")
                            nc.tensor.matmul(phd[:],
                                             lhsT=wskip_sb[:, i * DIM:(i + 1) * DIM],
                                             rhs=aggT[:], start=True, stop=False,
                                             skip_group_check=True)
                            bb_lo = blo + bb * BLK
                            nc.tensor.matmul(phd[:], lhsT=eye32_sb[:],
                                             rhs=hT[:, bb_lo:bb_lo + BLK],
                                             start=False, stop=True,
                                             skip_group_check=True)
                            tmp = wk.tile([128, 128], F32, tag="tmp")
                            nc.scalar.activation(tmp[:], phd[:], AF.Copy)
                            nc.vector.tensor_scalar_add(tmp[:], tmp[:],
                                                        bskip_sb[:, i:i + 1])
                            tmp2 = wk.tile([128, 128], F32, tag="tmp2")
                            nc.vector.tensor_scalar_mul(tmp2[:], tmp[:], SLOPE)
                            nc.vector.tensor_tensor(out=hT[:, bb_lo:bb_lo + BLK],
                                                    in0=tmp[:], in1=tmp2[:],
                                                    op=OP.max)
                            if i == L - 1:
                                b = b0 + bb
                                ptr = pt.tile([128, 128], F32, tag="pt")
                                nc.tensor.transpose(ptr[:], hT[:, bb_lo:bb_lo + BLK],
                                                    eye32_sb[:])
                                hrow = wk.tile([128, 128], BF16, tag="hrow")
                                nc.scalar.activation(hrow[:], ptr[:], AF.Copy)
                                mskb = wk.tile([128, 100], BF16, tag="mskb")
                                nc.sync.dma_start(mskb[:],
                                                  d_msk[:, b * 100:(b + 1) * 100])
                                nc.tensor.matmul(pooled_ps[:100, :], lhsT=mskb[:],
                                                 rhs=hrow[:], start=(b == 0),
                                                 stop=(b == BPC - 1),
                                                 skip_group_check=True)

            # ---- pooling allreduce + MLP ----
            pooled_sb = cst.tile([128, DIM], F32, tag="pooled")
            nc.vector.memset(pooled_sb[:], 0.0)
            nc.vector.tensor_copy(pooled_sb[:100, :], pooled_ps[:100, :])
            nc.sync.dma_start(ar_in[:], pooled_sb[:100, :])
            nc.gpsimd.collective_compute(
                "AllReduce", OP.add,
                ins=[ar_in.opt()], outs=[ar_out.opt()],
                replica_groups=[list(range(NCORES))],
            )
            nc.sync.dma_start(pooled_sb[:100, :], ar_out[:])
            invn_sb = cst.tile([128, 1], F32, tag="invn")
            nc.sync.dma_start(invn_sb[:], d_invn[:])
            nc.vector.tensor_scalar_mul(pooled_sb[:], pooled_sb[:], invn_sb[:, 0:1])

            w1_sb = cst.tile([128, DIM], F32, tag="w1")
            w2_sb = cst.tile([128, DIM], F32, tag="w2")
            w3_sb = cst.tile([128, 1], F32, tag="w3")
            b1_sb = cst.tile([128, DIM], F32, tag="b1")
            b2_sb = cst.tile([128, DIM], F32, tag="b2")
            b3_sb = cst.tile([128, 1], F32, tag="b3")
            nc.sync.dma_start(w1_sb[:], d_w1[:])
            nc.sync.dma_start(w2_sb[:], d_w2[:])
            nc.sync.dma_start(w3_sb[:], d_w3[:])
            nc.sync.dma_start(b1_sb[:], d_b1[:])
            nc.sync.dma_start(b2_sb[:], d_b2[:])
            nc.sync.dma_start(b3_sb[:], d_b3[:])

            def mlp_layer(src_sb, w_sb, b_sb, ncols):
                ptz = pt.tile([128, 128], F32, tag="pt")
                nc.tensor.transpose(ptz[:], src_sb[:], eye32_sb[:])
                srcT = wk.tile([128, 128], F32, tag="srcT")
                nc.vector.tensor_copy(srcT[:], ptz[:])
                pz = pa.tile([128, 2 * 144], F32, tag="acc")
                nc.tensor.matmul(pz[:100, :ncols], lhsT=srcT[:, 0:100],
                                 rhs=w_sb[:, :ncols], start=True, stop=True)
                zo = wk.tile([128, DIM], F32, tag="zo")
                nc.vector.memset(zo[:], 0.0)
                nc.vector.tensor_tensor(out=zo[:100, :ncols], in0=pz[:100, :ncols],
                                        in1=b_sb[:100, :ncols], op=OP.add)
                z2 = wk.tile([128, DIM], F32, tag="z2")
                nc.vector.memset(z2[:], 0.0)
                nc.vector.tensor_scalar_mul(z2[:100, :ncols], zo[:100, :ncols], SLOPE)
                nc.vector.tensor_tensor(out=zo[:100, :ncols], in0=zo[:100, :ncols],
                                        in1=z2[:100, :ncols], op=OP.max)
                return zo

            z1 = mlp_layer(pooled_sb, w1_sb, b1_sb, DIM)
            z1k = cst.tile([128, DIM], F32, tag="z1k")
            nc.vector.tensor_copy(z1k[:], z1[:])
            z2 = mlp_layer(z1k, w2_sb, b2_sb, DIM)
            z2k = cst.tile([128, DIM], F32, tag="z2k")
            nc.vector.tensor_copy(z2k[:], z2[:])
            ptz = pt.tile([128, 128], F32, tag="pt")
            nc.tensor.transpose(ptz[:], z2k[:], eye32_sb[:])
            zT = wk.tile([128, 128], F32, tag="srcT")
            nc.vector.tensor_copy(zT[:], ptz[:])
            po = pa.tile([128, 2 * 144], F32, tag="acc")
            nc.tensor.matmul(po[:100, 0:1], lhsT=zT[:, 0:100], rhs=w3_sb[:],
                             start=True, stop=True)
            outp = cst.tile([128, 1], F32, tag="outp")
            nc.vector.tensor_tensor(out=outp[:100, :], in0=po[:100, 0:1],
                                    in1=b3_sb[:100, :], op=OP.add)
            nc.sync.dma_start(d_out[:], outp[:100, :])

    nc.compile()
    return nc


def _wrap_idx(flat):
    """Lay out int16 gather indices in the Q7 wrap layout for one call."""
    n = flat.shape[0]
    arr = np.zeros((16, n // 16), np.int16)
    ii = np.arange(n)
    arr[ii % 16, ii // 16] = flat.astype(np.int16)
    return np.tile(arr, (8, 1))


def _preprocess(x, senders, receivers, n_node):
    """Build per-core input arrays."""
    order = np.argsort(receivers, kind="stable")
    r_s = receivers[order].astype(np.int64)
    s_s = senders[order].astype(np.int64)
    quarter = s_s // NQ

    graph_of = np.full(NPAD, -1, np.int64)
    graph_of[:N] = np.repeat(np.arange(G), n_node.astype(np.int64))

    arange128 = np.arange(128)
    per_core = []
    for c in range(NCORES):
        lo, hi = c * NPC, (c + 1) * NPC
        m = (r_s >= lo) & (r_s < hi)
        rc, sc, qc = r_s[m], s_s[m], quarter[m]
        blk = (rc - lo) // BLK
        key = blk * 4 + qc
        o2 = np.argsort(key, kind="stable")
        rc, sc, key = rc[o2], sc[o2], key[o2]
        qc = key % 4
        blk = key // 4
        counts = np.bincount(key, minlength=BPC * 4).reshape(BPC, 4)
        starts = np.zeros(BPC * 4 + 1, np.int64)
        np.cumsum(counts.reshape(-1), out=starts[1:])
        cap = TPQ * 128
        if counts.max() > cap:
            raise RuntimeError(f"(block,quarter) capacity exceeded: {counts.max()}")
        # slot arrays per (block, quarter)
        slot_s = np.zeros((BPC, 4, cap), np.int64)      # local sender row
        slot_r = np.full((BPC, 4, cap), 128, np.int64)  # r_rel (128 = pad)
        within = np.arange(len(key)) - starts[key]
        slot_s[blk, qc, within] = sc - qc * NQ
        slot_r[blk, qc, within] = rc - lo - blk * BLK
        # gather index array: call (g, q) = blocks of the group concatenated
        idx_arr = np.zeros((128, NCALLS * (NIDX // 16)), np.int16)
        for gg in range(NGRP):
            for qq in range(4):
                call = gg * 4 + qq
                flat = slot_s[gg * GPB:(gg + 1) * GPB, qq, :].reshape(-1)
                idx_arr[:, call * (NIDX // 16):(call + 1) * (NIDX // 16)] = _wrap_idx(flat)
        # indicator tiles; tile t = b*TPB + q*TPQ + j
        rr = slot_r.reshape(BPC, 4, TPQ, 128)                  # [b,q,j,e]
        ind = (rr[..., :, None] == arange128[None, None, None, None, :])
        ind_arr = np.ascontiguousarray(
            ind.transpose(3, 0, 1, 2, 4).reshape(128, TT * 128)).astype("bfloat16")
        ind2_arr = np.ascontiguousarray(
            ind.transpose(4, 0, 1, 2, 3).reshape(128, TT * 128)).astype("bfloat16")
        # pooling mask
        msk = np.zeros((128, BPC * 100), np.float32)
        nodes = np.arange(lo, hi)
        gg2 = graph_of[nodes].reshape(BPC, BLK)
        for bb in range(BPC):
            valid = gg2[bb] >= 0
            msk[arange128[:BLK][valid], bb * 100 + gg2[bb][valid]] = 1.0
        xT = np.zeros((F_IN, NPC), np.float32)
        nreal = max(0, min(NPC, N - lo))
        if nreal > 0:
            xT[:, :nreal] = x[lo:lo + nreal].T
        per_core.append(dict(
            xT=xT,
            idx=idx_arr,
            ind=ind_arr,
            ind2=ind2_arr,
            msk=msk.astype("bfloat16"),
        ))
    return per_core


def kernel(**inputs):
    global last_exec_time_ns
    x = np.asarray(inputs["x"], np.float32)
    senders = np.asarray(inputs["senders"])
    receivers = np.asarray(inputs["receivers"])
    n_node = np.asarray(inputs["n_node"])

    per_core = _preprocess(x, senders, receivers, n_node)

    W_in = np.asarray(inputs["W_in"], np.float32)
    b_in = np.asarray(inputs["b_in"], np.float32)
    W_gat = np.asarray(inputs["W_gat"], np.float32)
    a_src = np.asarray(inputs["a_src"], np.float32)
    a_dst = np.asarray(inputs["a_dst"], np.float32)
    W_skip = np.asarray(inputs["W_skip"], np.float32)
    b_skip = np.asarray(inputs["b_skip"], np.float32)
    W1 = np.asarray(inputs["W1"], np.float32)
    b1 = np.asarray(inputs["b1"], np.float32)
    W2 = np.asarray(inputs["W2"], np.float32)
    b2 = np.asarray(inputs["b2"], np.float32)
    W3 = np.asarray(inputs["W3"], np.float32)
    b3 = np.asarray(inputs["b3"], np.float32)

    def w_al(Wg, a):
        A = np.zeros((DIM, H), np.float32)
        for hh in range(H):
            A[hh * HD:(hh + 1) * HD, hh] = a[hh]
        return Wg @ A

    wcat = np.concatenate(
        [np.concatenate([W_gat[i], w_al(W_gat[i], a_src[i]),
                         w_al(W_gat[i], a_dst[i])], axis=1) for i in range(L)],
        axis=1)
    wskip = np.concatenate([W_skip[i] for i in range(L)], axis=1).astype("bfloat16")
    bskip = np.stack([b_skip[i] for i in range(L)], axis=1)

    eyebf = np.eye(128, dtype=np.float32).astype("bfloat16")
    eye32 = np.eye(128, dtype=np.float32)
    b1b = np.tile(b1, (128, 1)).astype(np.float32)
    b2b = np.tile(b2, (128, 1)).astype(np.float32)
    b3b = np.full((128, 1), float(b3[0]), np.float32)
    invn = np.ones((128, 1), np.float32)
    invn[:100, 0] = 1.0 / n_node.astype(np.float32)

    shared = dict(
        win=W_in, bin=b_in.reshape(DIM, 1), wcat=wcat, wskip=wskip, bskip=bskip,
        eyebf=eyebf, eye32=eye32,
        w1=W1, w2=W2, w3=W3.reshape(DIM, 1), b1b=b1b, b2b=b2b, b3b=b3b, invn=invn,
    )

    nc = _build_program()
    in_maps = [{**shared, **pc} for pc in per_core]
    trace = bool(int(os.environ.get("GAT_TRACE", "0")))
    res = run_bass_kernel_spmd(nc, in_maps, core_ids=list(range(NCORES)),
                               trace=trace)
    last_exec_time_ns = res.exec_time_ns
    out = np.asarray(res.results[0]["out"], np.float32).reshape(-1)
    return out
